# revision 7
# baseline (speedup 1.0000x reference)
"""MultiHeadDepthwiseSelfAttention TRN2 kernel v3 (8-core data-parallel over batch).

Structure (per core, 2 batches, software-pipelined emission):
- x loaded token-major (contiguous 3KB rows), PE-transposed into PSUM, then
  staged as padded x^T rows in SBUF (fast PSUM release).
- depthwise conv: center tap + bias alternate between PE (diagonal-weight
  matmul + ones-row bias matmul into PSUM) and ACT (activation scale+bias);
  the two shifted taps are DVE scalar_tensor_tensor; edge columns fall out
  of the zero padding.
- scores computed transposed (j on partitions); exp on ACT over 1024-wide
  PSUM tiles; attn^T via augmented V whose ones column sits at index 0 so
  the softmax denominator r lands on PSUM partition 0.
- 1/r broadcast across partitions with gpsimd partition_broadcast (no DMA,
  no PSUM); normalize multiply on DVE; SBUF->SBUF DMA restacks each head
  onto attnT partitions 0-63 / 64-127.
- output projection computed token-major (out[tok, feat]) so stores are
  natural contiguous DMAs; bo added via K=1 ones-row matmul accumulation.
- PSUM banks: xT/vT/mid/proj ring (2) + scores (4) + attn (2) = 8.
"""

import sys

sys.path.insert(0, "/opt/trn_rl_repo")

from contextlib import ExitStack

import numpy as np

import concourse.bass as bass
import concourse.tile as tile
from concourse import bacc, mybir
from concourse.masks import make_identity

F32 = mybir.dt.float32
F32R = mybir.dt.float32r

B, N, FEAT, HEAD, D, KS = 16, 512, 768, 12, 64, 3
NCORES = 8
B_LOC = B // NCORES          # batches per core
NCH = FEAT // 128            # 6 channel chunks (2 heads each)
NJB = N // 128               # 4 token blocks
MUL = mybir.AluOpType.mult
ADD = mybir.AluOpType.add

_PROG_CACHE = {}


def r32(ap):
    return ap.bitcast(F32R)


def build_program():
    if "nc" in _PROG_CACHE:
        return _PROG_CACHE["nc"]
    nc = bacc.Bacc("TRN2", target_bir_lowering=False)

    x_d = nc.dram_tensor("x", [B_LOC, N, FEAT], F32, kind="ExternalInput")
    wq_d = nc.dram_tensor("wq", [128, NCH, KS], F32, kind="ExternalInput")
    wk_d = nc.dram_tensor("wk", [128, NCH, KS], F32, kind="ExternalInput")
    wv_d = nc.dram_tensor("wv", [128, NCH, KS], F32, kind="ExternalInput")
    bq_d = nc.dram_tensor("bq", [128, NCH], F32, kind="ExternalInput")
    bk_d = nc.dram_tensor("bk", [128, NCH], F32, kind="ExternalInput")
    bv_d = nc.dram_tensor("bv", [128, NCH], F32, kind="ExternalInput")
    brow_d = nc.dram_tensor("brow", [3, FEAT], F32, kind="ExternalInput")
    woT_d = nc.dram_tensor("woT", [FEAT, FEAT], F32, kind="ExternalInput")
    bo_d = nc.dram_tensor("bo", [1, FEAT], F32, kind="ExternalInput")
    out_d = nc.dram_tensor("out", [B_LOC, N, FEAT], F32, kind="ExternalOutput")

    with tile.TileContext(nc) as tc, ExitStack() as ctx:
        consts = ctx.enter_context(tc.tile_pool(name="consts", bufs=1))
        x_pool = ctx.enter_context(tc.tile_pool(name="xtok", bufs=7))
        q_pool = ctx.enter_context(tc.tile_pool(name="qT", bufs=11))
        k_pool = ctx.enter_context(tc.tile_pool(name="kT", bufs=11))
        v_pool = ctx.enter_context(tc.tile_pool(name="vT", bufs=7))
        mid_pool = ctx.enter_context(tc.tile_pool(name="mid", bufs=3))
        xp_pool = ctx.enter_context(tc.tile_pool(name="xpad", bufs=4))
        va_pool = ctx.enter_context(tc.tile_pool(name="vaug", bufs=6))
        exp_pool = ctx.enter_context(tc.tile_pool(name="exp", bufs=3))
        rr_pool = ctx.enter_context(tc.tile_pool(name="rrow", bufs=3))
        brc_pool = ctx.enter_context(tc.tile_pool(name="brc", bufs=3))
        at_pool = ctx.enter_context(tc.tile_pool(name="attnT", bufs=7))
        st_pool = ctx.enter_context(tc.tile_pool(name="stk", bufs=2))
        ot_pool = ctx.enter_context(tc.tile_pool(name="outT", bufs=3))
        ps_xt = ctx.enter_context(tc.tile_pool(name="ps_xt", bufs=2, space="PSUM"))
        ps_sc = ctx.enter_context(tc.tile_pool(name="ps_sc", bufs=2, space="PSUM"))
        ps_at = ctx.enter_context(tc.tile_pool(name="ps_at", bufs=2, space="PSUM"))

        # ---- constants / weights ----
        ident = consts.tile([128, 128], F32)
        make_identity(nc, ident[:, :])
        ident_r = consts.tile([128, 128], F32)
        nc.vector.tensor_copy(out=r32(ident_r[:, :]), in_=ident[:, :])
        ones_plain = consts.tile([128, 512], F32)
        nc.vector.memset(ones_plain[:, :], 1.0)
        ones_row = consts.tile([1, 128], F32)
        nc.vector.tensor_copy(out=r32(ones_row[:, :]), in_=ones_plain[0:1, 0:128])

        wq_sb = consts.tile([128, NCH, KS], F32)
        wk_sb = consts.tile([128, NCH, KS], F32)
        wv_sb = consts.tile([128, NCH, KS], F32)
        bq_sb = consts.tile([128, NCH], F32)
        bk_sb = consts.tile([128, NCH], F32)
        bv_sb = consts.tile([128, NCH], F32)
        for sb, dr in ((wq_sb, wq_d), (wk_sb, wk_d), (wv_sb, wv_d),
                       (bq_sb, bq_d), (bk_sb, bk_d), (bv_sb, bv_d)):
            nc.sync.dma_start(out=sb[...], in_=dr.ap())
        woT_sb = []
        for fc in range(NCH):
            t = consts.tile([128, FEAT], F32, tag=f"woT{fc}")
            nc.scalar.dma_start(
                out=r32(t[:, :]), in_=r32(woT_d.ap()[fc * 128 : (fc + 1) * 128, :])
            )
            woT_sb.append(t)
        bo_row = consts.tile([1, FEAT], F32)
        nc.scalar.dma_start(out=r32(bo_row[:, :]), in_=r32(bo_d.ap()))
        ones512 = consts.tile([1, N], F32)
        nc.vector.tensor_copy(out=r32(ones512[:, :]), in_=ones_plain[0:1, :])
        wdiag_sb = []
        brow_sb = []
        for ch, w_sb in enumerate((wq_sb, wk_sb, wv_sb)):
            t = consts.tile([128, NCH, 128], F32, tag=f"wdiag{ch}")
            for c in range(NCH):
                nc.vector.tensor_scalar(
                    out=r32(t[:, c, :]), in0=ident[:, :],
                    scalar1=w_sb[:, c, 1:2], scalar2=None, op0=MUL,
                )
            wdiag_sb.append(t)
            tb_ = consts.tile([1, FEAT], F32, tag=f"brow{ch}")
            nc.scalar.dma_start(out=r32(tb_[:, :]), in_=r32(brow_d.ap()[ch : ch + 1, :]))
            brow_sb.append(tb_)

        x_ap = x_d.ap()
        out_ap = out_d.ap()

        # ---- token-major x loads (natural, contiguous), both batches ----
        x_all = []
        for b in range(B_LOC):
            for tb in range(NJB):
                xt = x_pool.tile([128, FEAT], F32)
                nc.sync.dma_start(
                    out=r32(xt[:, :]),
                    in_=r32(x_ap[b, tb * 128 : (tb + 1) * 128, :]),
                )
                x_all.append(xt)

        for b in range(B_LOC):
            x_tok = x_all[b * NJB : (b + 1) * NJB]

            # ---- per chunk: PE transpose into PSUM, stage padded x^T in SBUF ----
            qT, kT, vT = [], [], []
            for c in range(NCH):
                xps = ps_xt.tile([128, 512], F32, tag="sh")
                for tb in range(NJB):
                    nc.tensor.transpose(
                        out=r32(xps[:, tb * 128 : (tb + 1) * 128]),
                        in_=r32(x_tok[tb][:, c * 128 : (c + 1) * 128]),
                        identity=r32(ident_r[:, :]),
                    )
                xpad = xp_pool.tile([128, N + 2], F32)
                nc.gpsimd.memset(xpad[:, 0:1], 0.0)
                nc.gpsimd.memset(xpad[:, N + 1 : N + 2], 0.0)
                nc.scalar.copy(out=r32(xpad[:, 1 : N + 1]), in_=xps[:, :])
                qt = q_pool.tile([128, N], F32)
                kt = k_pool.tile([128, N], F32)
                vt = v_pool.tile([128, N], F32)
                for ch, (out_t, w_sb) in enumerate(((qt, wq_sb), (kt, wk_sb),
                                                    (vt, wv_sb))):
                    # center tap + bias on PE (diag matmul + ones-row matmul)
                    mps = ps_xt.tile([128, 512], F32, tag="sh")
                    nc.tensor.matmul(
                        out=mps[:, :],
                        lhsT=r32(wdiag_sb[ch][:, c, :]),
                        rhs=r32(xpad[:, 1 : N + 1]),
                        start=True,
                        stop=False,
                    )
                    nc.tensor.matmul(
                        out=mps[:, :],
                        lhsT=r32(brow_sb[ch][:, c * 128 : (c + 1) * 128]),
                        rhs=r32(ones512[:, :]),
                        start=False,
                        stop=True,
                    )
                    # the two shifted taps (DVE), first reads PSUM mid
                    mid = mid_pool.tile([128, N], F32)
                    nc.vector.scalar_tensor_tensor(
                        out=mid[:, :], in0=xpad[:, 0:N],
                        scalar=w_sb[:, c, 0:1], in1=mps[:, :],
                        op0=MUL, op1=ADD,
                    )
                    nc.vector.scalar_tensor_tensor(
                        out=r32(out_t[:, :]), in0=xpad[:, 2 : N + 2],
                        scalar=w_sb[:, c, 2:3], in1=mid[:, :],
                        op0=MUL, op1=ADD,
                    )
                qT.append(qt)
                kT.append(kt)
                vT.append(vt)

            # ---- v to token-major augmented tiles (ones col 0 -> r row 0) ----
            v_aug = []
            for jb in range(NJB):
                va = va_pool.tile([128, HEAD, D + 1], F32)
                for g in range(2):
                    vps = ps_xt.tile([128, 512], F32, tag="sh")
                    for kk in range(3):
                        cc = 3 * g + kk
                        nc.tensor.transpose(
                            out=r32(vps[:, kk * 128 : (kk + 1) * 128]),
                            in_=r32(vT[cc][:, jb * 128 : (jb + 1) * 128]),
                            identity=r32(ident_r[:, :]),
                        )
                    nc.scalar.copy(
                        out=r32(va[:, 6 * g : 6 * g + 6, 1 : D + 1]),
                        in_=vps[:, 0:384].rearrange("p (h d) -> p h d", h=6),
                    )
                nc.scalar.copy(
                    out=r32(va[:, :, 0:1]),
                    in_=ones_plain[:, 0:HEAD].rearrange("p (h o) -> p h o", o=1),
                )
                v_aug.append(va)

            # ---- attention per head pair ----
            attnT = []
            for pair in range(NCH):
                at = at_pool.tile([128, N], F32)
                for half in (0, 1):
                    h = 2 * pair + half
                    hp = slice(64 * half, 64 * half + 64)
                    exs = []
                    for sg in range(2):
                        sc = ps_sc.tile([128, 1024], F32, tag="sc")
                        for jj in range(2):
                            jb = 2 * sg + jj
                            nc.tensor.matmul(
                                out=sc[:, jj * 512 : (jj + 1) * 512],
                                lhsT=r32(kT[pair][hp, jb * 128 : (jb + 1) * 128]),
                                rhs=r32(qT[pair][hp, :]),
                                start=True,
                                stop=True,
                            )
                        ex = exp_pool.tile([128, 2, N], F32)
                        nc.scalar.activation(
                            out=r32(ex[:, :, :]),
                            in_=sc[:, :].rearrange("p (a b) -> p a b", a=2),
                            func=mybir.ActivationFunctionType.Exp,
                        )
                        exs.append(ex)
                    # attn^T rows 1..64; ones column makes row 0 = r
                    aps = ps_at.tile([D + 1, 512], F32, tag="at")
                    for jc in range(NJB):
                        nc.tensor.matmul(
                            out=aps[:, :],
                            lhsT=r32(v_aug[jc][:, h, :]),
                            rhs=r32(exs[jc // 2][:, jc % 2, :]),
                            start=(jc == 0),
                            stop=(jc == NJB - 1),
                        )
                    # 1/r on partition 0, gpsimd broadcast, normalize, restack
                    rr = rr_pool.tile([1, 512], F32)
                    with nc.allow_low_precision(reason="softmax denominator"):
                        nc.vector.reciprocal(out=rr[:, :], in_=aps[0:1, :])
                    brc = brc_pool.tile([D + 1, 512], F32)
                    nc.gpsimd.partition_broadcast(brc[:, :], rr[:, :])
                    stk = st_pool.tile([D + 1, N], F32)
                    nc.vector.tensor_tensor(
                        out=r32(stk[:, :]), in0=aps[:, :],
                        in1=brc[:, :], op=MUL,
                    )
                    nc.sync.dma_start(
                        out=r32(at[64 * half : 64 * half + 64, :]),
                        in_=r32(stk[1 : D + 1, :]),
                    )
                attnT.append(at)

            # ---- output projection, token-major + bias, natural store ----
            for tb in range(NJB):
                ot = ot_pool.tile([128, FEAT], F32)
                for gh in range(2):
                    pj = ps_xt.tile([128, 512], F32, tag="sh")
                    gs = slice(gh * 384, gh * 384 + 384)
                    for fc in range(NCH):
                        nc.tensor.matmul(
                            out=pj[:, 0:384],
                            lhsT=r32(attnT[fc][:, tb * 128 : (tb + 1) * 128]),
                            rhs=r32(woT_sb[fc][:, gs]),
                            start=(fc == 0),
                            stop=False,
                        )
                    # += broadcast bias row (K=1 ones-row matmul)
                    nc.tensor.matmul(
                        out=pj[:, 0:384],
                        lhsT=r32(ones_row[:, :]),
                        rhs=r32(bo_row[:, gs]),
                        start=False,
                        stop=True,
                    )
                    nc.scalar.copy(out=ot[:, gs], in_=pj[:, 0:384])
                nc.sync.dma_start(
                    out=out_ap[b, tb * 128 : (tb + 1) * 128, :], in_=ot[:, :]
                )

    nc.compile()
    _PROG_CACHE["nc"] = nc
    return nc


def host_inputs(x, wq, bq, wk, bk, wv, bv, wo, bo):
    """Per-core input maps. Weight layout transforms + 1/sqrt(F) fold into q."""
    s = 1.0 / np.sqrt(np.float32(FEAT))

    def taps(w):  # (F,1,K) -> (128, NCH, K)
        return np.ascontiguousarray(
            w[:, 0, :].reshape(NCH, 128, KS).transpose(1, 0, 2)
        ).astype(np.float32)

    def cols(v):  # (F,) -> (128, NCH)
        return np.ascontiguousarray(v.reshape(NCH, 128).T).astype(np.float32)

    brow = np.stack([
        (bq * s).astype(np.float32), bk.astype(np.float32), bv.astype(np.float32)
    ]).reshape(3, FEAT)
    shared = {
        "wq": taps(wq) * s, "bq": cols(bq) * s,
        "wk": taps(wk), "bk": cols(bk),
        "wv": taps(wv), "bv": cols(bv),
        "brow": np.ascontiguousarray(brow).astype(np.float32),
        "woT": np.ascontiguousarray(wo.T).astype(np.float32),
        "bo": np.ascontiguousarray(bo.reshape(1, FEAT)).astype(np.float32),
    }
    return [
        {"x": np.ascontiguousarray(x[c * B_LOC : (c + 1) * B_LOC]).astype(np.float32),
         **shared}
        for c in range(NCORES)
    ]


def kernel(x, wq, bq, wk, bk, wv, bv, wo, bo):
    from concourse.bass_utils import run_bass_kernel_spmd

    nc = build_program()
    x = np.asarray(x)
    in_maps = host_inputs(
        x, np.asarray(wq), np.asarray(bq), np.asarray(wk), np.asarray(bk),
        np.asarray(wv), np.asarray(bv), np.asarray(wo), np.asarray(bo),
    )
    res = run_bass_kernel_spmd(nc, in_maps, list(range(NCORES)))
    out = np.concatenate([res.results[c]["out"] for c in range(NCORES)], axis=0)
    return out.astype(np.float32)


# revision 8
# speedup vs baseline: 1.0127x; 1.0127x over previous
"""MultiHeadDepthwiseSelfAttention TRN2 kernel v3 (8-core data-parallel over batch).

Structure (per core, 2 batches, software-pipelined emission):
- x loaded token-major (contiguous 3KB rows), PE-transposed into PSUM, then
  staged as padded x^T rows in SBUF (fast PSUM release).
- depthwise conv: center tap + bias alternate between PE (diagonal-weight
  matmul + ones-row bias matmul into PSUM) and ACT (activation scale+bias);
  the two shifted taps are DVE scalar_tensor_tensor; edge columns fall out
  of the zero padding.
- scores computed transposed (j on partitions); exp on ACT over 1024-wide
  PSUM tiles; attn^T via augmented V whose ones column sits at index 0 so
  the softmax denominator r lands on PSUM partition 0.
- 1/r broadcast across partitions with gpsimd partition_broadcast (no DMA,
  no PSUM); normalize multiply on DVE; SBUF->SBUF DMA restacks each head
  onto attnT partitions 0-63 / 64-127.
- output projection computed token-major (out[tok, feat]) so stores are
  natural contiguous DMAs; bo added via K=1 ones-row matmul accumulation.
- PSUM banks: xT/vT/mid/proj ring (2) + scores (4) + attn (2) = 8.
"""

import sys

sys.path.insert(0, "/opt/trn_rl_repo")

from contextlib import ExitStack

import numpy as np

import concourse.bass as bass
import concourse.tile as tile
from concourse import bacc, mybir
from concourse.masks import make_identity

F32 = mybir.dt.float32
F32R = mybir.dt.float32r

B, N, FEAT, HEAD, D, KS = 16, 512, 768, 12, 64, 3
NCORES = 8
B_LOC = B // NCORES          # batches per core
NCH = FEAT // 128            # 6 channel chunks (2 heads each)
NJB = N // 128               # 4 token blocks
MUL = mybir.AluOpType.mult
ADD = mybir.AluOpType.add

_PROG_CACHE = {}


def r32(ap):
    return ap.bitcast(F32R)


def build_program():
    if "nc" in _PROG_CACHE:
        return _PROG_CACHE["nc"]
    nc = bacc.Bacc("TRN2", target_bir_lowering=False)

    x_d = nc.dram_tensor("x", [B_LOC, N, FEAT], F32, kind="ExternalInput")
    wq_d = nc.dram_tensor("wq", [128, NCH, KS], F32, kind="ExternalInput")
    wk_d = nc.dram_tensor("wk", [128, NCH, KS], F32, kind="ExternalInput")
    wv_d = nc.dram_tensor("wv", [128, NCH, KS], F32, kind="ExternalInput")
    bq_d = nc.dram_tensor("bq", [128, NCH], F32, kind="ExternalInput")
    bk_d = nc.dram_tensor("bk", [128, NCH], F32, kind="ExternalInput")
    bv_d = nc.dram_tensor("bv", [128, NCH], F32, kind="ExternalInput")
    brow_d = nc.dram_tensor("brow", [3, FEAT], F32, kind="ExternalInput")
    woT_d = nc.dram_tensor("woT", [FEAT, FEAT], F32, kind="ExternalInput")
    bo_d = nc.dram_tensor("bo", [1, FEAT], F32, kind="ExternalInput")
    out_d = nc.dram_tensor("out", [B_LOC, N, FEAT], F32, kind="ExternalOutput")

    with tile.TileContext(nc) as tc, ExitStack() as ctx:
        consts = ctx.enter_context(tc.tile_pool(name="consts", bufs=1))
        x_pool = ctx.enter_context(tc.tile_pool(name="xtok", bufs=7))
        q_pool = ctx.enter_context(tc.tile_pool(name="qT", bufs=11))
        k_pool = ctx.enter_context(tc.tile_pool(name="kT", bufs=11))
        v_pool = ctx.enter_context(tc.tile_pool(name="vT", bufs=7))
        mid_pool = ctx.enter_context(tc.tile_pool(name="mid", bufs=3))
        xp_pool = ctx.enter_context(tc.tile_pool(name="xpad", bufs=4))
        va_pool = ctx.enter_context(tc.tile_pool(name="vaug", bufs=6))
        exp_pool = ctx.enter_context(tc.tile_pool(name="exp", bufs=3))
        rr_pool = ctx.enter_context(tc.tile_pool(name="rrow", bufs=3))
        brc_pool = ctx.enter_context(tc.tile_pool(name="brc", bufs=3))
        at_pool = ctx.enter_context(tc.tile_pool(name="attnT", bufs=7))
        st_pool = ctx.enter_context(tc.tile_pool(name="stk", bufs=2))
        ot_pool = ctx.enter_context(tc.tile_pool(name="outT", bufs=3))
        ps_xt = ctx.enter_context(tc.tile_pool(name="ps_xt", bufs=2, space="PSUM"))
        ps_sc = ctx.enter_context(tc.tile_pool(name="ps_sc", bufs=2, space="PSUM"))
        ps_at = ctx.enter_context(tc.tile_pool(name="ps_at", bufs=2, space="PSUM"))

        # ---- constants / weights ----
        ident = consts.tile([128, 128], F32)
        make_identity(nc, ident[:, :])
        ident_r = consts.tile([128, 128], F32)
        nc.vector.tensor_copy(out=r32(ident_r[:, :]), in_=ident[:, :])
        ones_plain = consts.tile([128, 512], F32)
        nc.vector.memset(ones_plain[:, :], 1.0)
        ones_row = consts.tile([1, 128], F32)
        nc.vector.tensor_copy(out=r32(ones_row[:, :]), in_=ones_plain[0:1, 0:128])

        wq_sb = consts.tile([128, NCH, KS], F32)
        wk_sb = consts.tile([128, NCH, KS], F32)
        wv_sb = consts.tile([128, NCH, KS], F32)
        bq_sb = consts.tile([128, NCH], F32)
        bk_sb = consts.tile([128, NCH], F32)
        bv_sb = consts.tile([128, NCH], F32)
        for sb, dr in ((wq_sb, wq_d), (wk_sb, wk_d), (wv_sb, wv_d),
                       (bq_sb, bq_d), (bk_sb, bk_d), (bv_sb, bv_d)):
            nc.sync.dma_start(out=sb[...], in_=dr.ap())
        woT_sb = []
        for fc in range(NCH):
            t = consts.tile([128, FEAT], F32, tag=f"woT{fc}")
            nc.scalar.dma_start(
                out=r32(t[:, :]), in_=r32(woT_d.ap()[fc * 128 : (fc + 1) * 128, :])
            )
            woT_sb.append(t)
        bo_row = consts.tile([1, FEAT], F32)
        nc.scalar.dma_start(out=r32(bo_row[:, :]), in_=r32(bo_d.ap()))
        ones512 = consts.tile([1, N], F32)
        nc.vector.tensor_copy(out=r32(ones512[:, :]), in_=ones_plain[0:1, :])
        wdiag_sb = []
        brow_sb = []
        for ch, w_sb in enumerate((wq_sb, wk_sb, wv_sb)):
            t = consts.tile([128, NCH, 128], F32, tag=f"wdiag{ch}")
            for c in range(NCH):
                nc.vector.tensor_scalar(
                    out=r32(t[:, c, :]), in0=ident[:, :],
                    scalar1=w_sb[:, c, 1:2], scalar2=None, op0=MUL,
                )
            wdiag_sb.append(t)
            tb_ = consts.tile([1, FEAT], F32, tag=f"brow{ch}")
            nc.scalar.dma_start(out=r32(tb_[:, :]), in_=r32(brow_d.ap()[ch : ch + 1, :]))
            brow_sb.append(tb_)

        x_ap = x_d.ap()
        out_ap = out_d.ap()

        # ---- token-major x loads (natural, contiguous), both batches ----
        x_all = []
        for b in range(B_LOC):
            for tb in range(NJB):
                xt = x_pool.tile([128, FEAT], F32)
                nc.sync.dma_start(
                    out=r32(xt[:, :]),
                    in_=r32(x_ap[b, tb * 128 : (tb + 1) * 128, :]),
                )
                x_all.append(xt)

        for b in range(B_LOC):
            x_tok = x_all[b * NJB : (b + 1) * NJB]

            # ---- per chunk: PE transpose into PSUM, stage padded x^T in SBUF ----
            qT, kT, vT = [], [], []
            for c in range(NCH):
                xps = ps_xt.tile([128, 512], F32, tag="sh")
                for tb in range(NJB):
                    nc.tensor.transpose(
                        out=r32(xps[:, tb * 128 : (tb + 1) * 128]),
                        in_=r32(x_tok[tb][:, c * 128 : (c + 1) * 128]),
                        identity=r32(ident_r[:, :]),
                    )
                xpad = xp_pool.tile([128, N + 2], F32)
                nc.gpsimd.memset(xpad[:, 0:1], 0.0)
                nc.gpsimd.memset(xpad[:, N + 1 : N + 2], 0.0)
                nc.scalar.copy(out=r32(xpad[:, 1 : N + 1]), in_=xps[:, :])
                qt = q_pool.tile([128, N], F32)
                kt = k_pool.tile([128, N], F32)
                vt = v_pool.tile([128, N], F32)
                for ch, (out_t, w_sb) in enumerate(((qt, wq_sb), (kt, wk_sb),
                                                    (vt, wv_sb))):
                    # center tap + bias on PE (diag matmul + ones-row matmul)
                    mps = ps_xt.tile([128, 512], F32, tag="sh")
                    nc.tensor.matmul(
                        out=mps[:, :],
                        lhsT=r32(wdiag_sb[ch][:, c, :]),
                        rhs=r32(xpad[:, 1 : N + 1]),
                        start=True,
                        stop=False,
                    )
                    nc.tensor.matmul(
                        out=mps[:, :],
                        lhsT=r32(brow_sb[ch][:, c * 128 : (c + 1) * 128]),
                        rhs=r32(ones512[:, :]),
                        start=False,
                        stop=True,
                    )
                    # the two shifted taps (DVE), first reads PSUM mid
                    mid = mid_pool.tile([128, N], F32)
                    nc.vector.scalar_tensor_tensor(
                        out=mid[:, :], in0=xpad[:, 0:N],
                        scalar=w_sb[:, c, 0:1], in1=mps[:, :],
                        op0=MUL, op1=ADD,
                    )
                    nc.vector.scalar_tensor_tensor(
                        out=r32(out_t[:, :]), in0=xpad[:, 2 : N + 2],
                        scalar=w_sb[:, c, 2:3], in1=mid[:, :],
                        op0=MUL, op1=ADD,
                    )
                qT.append(qt)
                kT.append(kt)
                vT.append(vt)

            # ---- v to token-major augmented tiles (ones col 0 -> r row 0) ----
            v_aug = []
            for jb in range(NJB):
                va = va_pool.tile([128, HEAD, D + 1], F32)
                for g in range(2):
                    vps = ps_xt.tile([128, 512], F32, tag="sh")
                    for kk in range(3):
                        cc = 3 * g + kk
                        nc.tensor.transpose(
                            out=r32(vps[:, kk * 128 : (kk + 1) * 128]),
                            in_=r32(vT[cc][:, jb * 128 : (jb + 1) * 128]),
                            identity=r32(ident_r[:, :]),
                        )
                    nc.scalar.copy(
                        out=r32(va[:, 6 * g : 6 * g + 6, 1 : D + 1]),
                        in_=vps[:, 0:384].rearrange("p (h d) -> p h d", h=6),
                    )
                nc.scalar.copy(
                    out=r32(va[:, :, 0:1]),
                    in_=ones_plain[:, 0:HEAD].rearrange("p (h o) -> p h o", o=1),
                )
                v_aug.append(va)

            # ---- attention per head pair ----
            attnT = []
            for pair in range(NCH):
                at = at_pool.tile([128, N], F32)
                for half in (0, 1):
                    h = 2 * pair + half
                    hp = slice(64 * half, 64 * half + 64)
                    exs = []
                    for sg in range(2):
                        sc = ps_sc.tile([128, 1024], F32, tag="sc")
                        for jj in range(2):
                            jb = 2 * sg + jj
                            nc.tensor.matmul(
                                out=sc[:, jj * 512 : (jj + 1) * 512],
                                lhsT=r32(kT[pair][hp, jb * 128 : (jb + 1) * 128]),
                                rhs=r32(qT[pair][hp, :]),
                                start=True,
                                stop=True,
                            )
                        ex = exp_pool.tile([128, 2, N], F32)
                        nc.scalar.activation(
                            out=r32(ex[:, :, :]),
                            in_=sc[:, :].rearrange("p (a b) -> p a b", a=2),
                            func=mybir.ActivationFunctionType.Exp,
                        )
                        exs.append(ex)
                    # attn^T rows 1..64; ones column makes row 0 = r
                    aps = ps_at.tile([D + 1, 512], F32)
                    for jc in range(NJB):
                        nc.tensor.matmul(
                            out=aps[:, :],
                            lhsT=r32(v_aug[jc][:, h, :]),
                            rhs=r32(exs[jc // 2][:, jc % 2, :]),
                            start=(jc == 0),
                            stop=(jc == NJB - 1),
                        )
                    # 1/r on partition 0, gpsimd broadcast, normalize, restack
                    rr = rr_pool.tile([1, 512], F32)
                    with nc.allow_low_precision(reason="softmax denominator"):
                        nc.vector.reciprocal(out=rr[:, :], in_=aps[0:1, :])
                    brc = brc_pool.tile([D + 1, 512], F32)
                    nc.gpsimd.partition_broadcast(brc[:, :], rr[:, :])
                    stk = st_pool.tile([D + 1, N], F32)
                    nc.vector.tensor_tensor(
                        out=r32(stk[:, :]), in0=aps[:, :],
                        in1=brc[:, :], op=MUL,
                    )
                    nc.sync.dma_start(
                        out=r32(at[64 * half : 64 * half + 64, :]),
                        in_=r32(stk[1 : D + 1, :]),
                    )
                attnT.append(at)

            # ---- output projection, token-major + bias, natural store ----
            for tb in range(NJB):
                ot = ot_pool.tile([128, FEAT], F32)
                for gh in range(2):
                    pj = ps_xt.tile([128, 512], F32, tag="sh")
                    gs = slice(gh * 384, gh * 384 + 384)
                    for fc in range(NCH):
                        nc.tensor.matmul(
                            out=pj[:, 0:384],
                            lhsT=r32(attnT[fc][:, tb * 128 : (tb + 1) * 128]),
                            rhs=r32(woT_sb[fc][:, gs]),
                            start=(fc == 0),
                            stop=False,
                        )
                    # += broadcast bias row (K=1 ones-row matmul)
                    nc.tensor.matmul(
                        out=pj[:, 0:384],
                        lhsT=r32(ones_row[:, :]),
                        rhs=r32(bo_row[:, gs]),
                        start=False,
                        stop=True,
                    )
                    nc.scalar.copy(out=ot[:, gs], in_=pj[:, 0:384])
                nc.sync.dma_start(
                    out=out_ap[b, tb * 128 : (tb + 1) * 128, :], in_=ot[:, :]
                )

    nc.compile()
    _PROG_CACHE["nc"] = nc
    return nc


def host_inputs(x, wq, bq, wk, bk, wv, bv, wo, bo):
    """Per-core input maps. Weight layout transforms + 1/sqrt(F) fold into q."""
    s = 1.0 / np.sqrt(np.float32(FEAT))

    def taps(w):  # (F,1,K) -> (128, NCH, K)
        return np.ascontiguousarray(
            w[:, 0, :].reshape(NCH, 128, KS).transpose(1, 0, 2)
        ).astype(np.float32)

    def cols(v):  # (F,) -> (128, NCH)
        return np.ascontiguousarray(v.reshape(NCH, 128).T).astype(np.float32)

    brow = np.stack([
        (bq * s).astype(np.float32), bk.astype(np.float32), bv.astype(np.float32)
    ]).reshape(3, FEAT)
    shared = {
        "wq": taps(wq) * s, "bq": cols(bq) * s,
        "wk": taps(wk), "bk": cols(bk),
        "wv": taps(wv), "bv": cols(bv),
        "brow": np.ascontiguousarray(brow).astype(np.float32),
        "woT": np.ascontiguousarray(wo.T).astype(np.float32),
        "bo": np.ascontiguousarray(bo.reshape(1, FEAT)).astype(np.float32),
    }
    return [
        {"x": np.ascontiguousarray(x[c * B_LOC : (c + 1) * B_LOC]).astype(np.float32),
         **shared}
        for c in range(NCORES)
    ]


def kernel(x, wq, bq, wk, bk, wv, bv, wo, bo):
    from concourse.bass_utils import run_bass_kernel_spmd

    nc = build_program()
    x = np.asarray(x)
    in_maps = host_inputs(
        x, np.asarray(wq), np.asarray(bq), np.asarray(wk), np.asarray(bk),
        np.asarray(wv), np.asarray(bv), np.asarray(wo), np.asarray(bo),
    )
    res = run_bass_kernel_spmd(nc, in_maps, list(range(NCORES)))
    out = np.concatenate([res.results[c]["out"] for c in range(NCORES)], axis=0)
    return out.astype(np.float32)


# revision 9
# speedup vs baseline: 1.0191x; 1.0063x over previous
"""MultiHeadDepthwiseSelfAttention TRN2 kernel v3 (8-core data-parallel over batch).

Structure (per core, 2 batches, software-pipelined emission):
- x loaded token-major (contiguous 3KB rows), PE-transposed into PSUM, then
  staged as padded x^T rows in SBUF (fast PSUM release).
- depthwise conv: center tap + bias alternate between PE (diagonal-weight
  matmul + ones-row bias matmul into PSUM) and ACT (activation scale+bias);
  the two shifted taps are DVE scalar_tensor_tensor; edge columns fall out
  of the zero padding.
- scores computed transposed (j on partitions); exp on ACT over 1024-wide
  PSUM tiles; attn^T via augmented V whose ones column sits at index 0 so
  the softmax denominator r lands on PSUM partition 0.
- 1/r broadcast across partitions with gpsimd partition_broadcast (no DMA,
  no PSUM); normalize multiply on DVE; SBUF->SBUF DMA restacks each head
  onto attnT partitions 0-63 / 64-127.
- output projection computed token-major (out[tok, feat]) so stores are
  natural contiguous DMAs; bo added via K=1 ones-row matmul accumulation.
- PSUM banks: xT/vT/mid/proj ring (2) + scores (4) + attn (2) = 8.
"""

import sys

sys.path.insert(0, "/opt/trn_rl_repo")

from contextlib import ExitStack

import numpy as np

import concourse.bass as bass
import concourse.tile as tile
from concourse import bacc, mybir
from concourse.masks import make_identity

F32 = mybir.dt.float32
F32R = mybir.dt.float32r

B, N, FEAT, HEAD, D, KS = 16, 512, 768, 12, 64, 3
NCORES = 8
B_LOC = B // NCORES          # batches per core
NCH = FEAT // 128            # 6 channel chunks (2 heads each)
NJB = N // 128               # 4 token blocks
MUL = mybir.AluOpType.mult
ADD = mybir.AluOpType.add

_PROG_CACHE = {}


def r32(ap):
    return ap.bitcast(F32R)


def build_program():
    if "nc" in _PROG_CACHE:
        return _PROG_CACHE["nc"]
    nc = bacc.Bacc("TRN2", target_bir_lowering=False)

    x_d = nc.dram_tensor("x", [B_LOC, N, FEAT], F32, kind="ExternalInput")
    wq_d = nc.dram_tensor("wq", [128, NCH, KS], F32, kind="ExternalInput")
    wk_d = nc.dram_tensor("wk", [128, NCH, KS], F32, kind="ExternalInput")
    wv_d = nc.dram_tensor("wv", [128, NCH, KS], F32, kind="ExternalInput")
    bq_d = nc.dram_tensor("bq", [128, NCH], F32, kind="ExternalInput")
    bk_d = nc.dram_tensor("bk", [128, NCH], F32, kind="ExternalInput")
    bv_d = nc.dram_tensor("bv", [128, NCH], F32, kind="ExternalInput")
    brow_d = nc.dram_tensor("brow", [3, FEAT], F32, kind="ExternalInput")
    woT_d = nc.dram_tensor("woT", [FEAT, FEAT], F32, kind="ExternalInput")
    bo_d = nc.dram_tensor("bo", [1, FEAT], F32, kind="ExternalInput")
    out_d = nc.dram_tensor("out", [B_LOC, N, FEAT], F32, kind="ExternalOutput")

    with tile.TileContext(nc) as tc, ExitStack() as ctx:
        consts = ctx.enter_context(tc.tile_pool(name="consts", bufs=1))
        x_pool = ctx.enter_context(tc.tile_pool(name="xtok", bufs=7))
        q_pool = ctx.enter_context(tc.tile_pool(name="qT", bufs=11))
        k_pool = ctx.enter_context(tc.tile_pool(name="kT", bufs=11))
        v_pool = ctx.enter_context(tc.tile_pool(name="vT", bufs=7))
        mid_pool = ctx.enter_context(tc.tile_pool(name="mid", bufs=3))
        xp_pool = ctx.enter_context(tc.tile_pool(name="xpad", bufs=4))
        va_pool = ctx.enter_context(tc.tile_pool(name="vaug", bufs=5))
        exp_pool = ctx.enter_context(tc.tile_pool(name="exp", bufs=4))
        rr_pool = ctx.enter_context(tc.tile_pool(name="rrow", bufs=3))
        brc_pool = ctx.enter_context(tc.tile_pool(name="brc", bufs=2))
        at_pool = ctx.enter_context(tc.tile_pool(name="attnT", bufs=7))
        st_pool = ctx.enter_context(tc.tile_pool(name="stk", bufs=2))
        ot_pool = ctx.enter_context(tc.tile_pool(name="outT", bufs=3))
        ps_xt = ctx.enter_context(tc.tile_pool(name="ps_xt", bufs=2, space="PSUM"))
        ps_sc = ctx.enter_context(tc.tile_pool(name="ps_sc", bufs=2, space="PSUM"))
        ps_at = ctx.enter_context(tc.tile_pool(name="ps_at", bufs=2, space="PSUM"))

        # ---- constants / weights ----
        ident = consts.tile([128, 128], F32)
        make_identity(nc, ident[:, :])
        ident_r = consts.tile([128, 128], F32)
        nc.vector.tensor_copy(out=r32(ident_r[:, :]), in_=ident[:, :])
        ones_plain = consts.tile([128, 512], F32)
        nc.vector.memset(ones_plain[:, :], 1.0)
        ones_row = consts.tile([1, 128], F32)
        nc.vector.tensor_copy(out=r32(ones_row[:, :]), in_=ones_plain[0:1, 0:128])

        wq_sb = consts.tile([128, NCH, KS], F32)
        wk_sb = consts.tile([128, NCH, KS], F32)
        wv_sb = consts.tile([128, NCH, KS], F32)
        bq_sb = consts.tile([128, NCH], F32)
        bk_sb = consts.tile([128, NCH], F32)
        bv_sb = consts.tile([128, NCH], F32)
        for sb, dr in ((wq_sb, wq_d), (wk_sb, wk_d), (wv_sb, wv_d),
                       (bq_sb, bq_d), (bk_sb, bk_d), (bv_sb, bv_d)):
            nc.sync.dma_start(out=sb[...], in_=dr.ap())
        woT_sb = []
        for fc in range(NCH):
            t = consts.tile([128, FEAT], F32, tag=f"woT{fc}")
            nc.scalar.dma_start(
                out=r32(t[:, :]), in_=r32(woT_d.ap()[fc * 128 : (fc + 1) * 128, :])
            )
            woT_sb.append(t)
        bo_row = consts.tile([1, FEAT], F32)
        nc.scalar.dma_start(out=r32(bo_row[:, :]), in_=r32(bo_d.ap()))
        ones512 = consts.tile([1, N], F32)
        nc.vector.tensor_copy(out=r32(ones512[:, :]), in_=ones_plain[0:1, :])
        wdiag_sb = []
        brow_sb = []
        for ch, w_sb in enumerate((wq_sb, wk_sb, wv_sb)):
            t = consts.tile([128, NCH, 128], F32, tag=f"wdiag{ch}")
            for c in range(NCH):
                nc.vector.tensor_scalar(
                    out=r32(t[:, c, :]), in0=ident[:, :],
                    scalar1=w_sb[:, c, 1:2], scalar2=None, op0=MUL,
                )
            wdiag_sb.append(t)
            tb_ = consts.tile([1, FEAT], F32, tag=f"brow{ch}")
            nc.scalar.dma_start(out=r32(tb_[:, :]), in_=r32(brow_d.ap()[ch : ch + 1, :]))
            brow_sb.append(tb_)

        x_ap = x_d.ap()
        out_ap = out_d.ap()

        # ---- token-major x loads (natural, contiguous), both batches ----
        x_all = []
        for b in range(B_LOC):
            for tb in range(NJB):
                xt = x_pool.tile([128, FEAT], F32)
                nc.sync.dma_start(
                    out=r32(xt[:, :]),
                    in_=r32(x_ap[b, tb * 128 : (tb + 1) * 128, :]),
                )
                x_all.append(xt)

        for b in range(B_LOC):
            x_tok = x_all[b * NJB : (b + 1) * NJB]

            # ---- per chunk: PE transpose into PSUM, stage padded x^T in SBUF ----
            qT, kT, vT = [], [], []
            for c in range(NCH):
                xps = ps_xt.tile([128, 512], F32, tag="sh")
                for tb in range(NJB):
                    nc.tensor.transpose(
                        out=r32(xps[:, tb * 128 : (tb + 1) * 128]),
                        in_=r32(x_tok[tb][:, c * 128 : (c + 1) * 128]),
                        identity=r32(ident_r[:, :]),
                    )
                xpad = xp_pool.tile([128, N + 2], F32)
                nc.gpsimd.memset(xpad[:, 0:1], 0.0)
                nc.gpsimd.memset(xpad[:, N + 1 : N + 2], 0.0)
                nc.scalar.copy(out=r32(xpad[:, 1 : N + 1]), in_=xps[:, :])
                qt = q_pool.tile([128, N], F32)
                kt = k_pool.tile([128, N], F32)
                vt = v_pool.tile([128, N], F32)
                for ch, (out_t, w_sb) in enumerate(((qt, wq_sb), (kt, wk_sb),
                                                    (vt, wv_sb))):
                    # center tap + bias on PE (diag matmul + ones-row matmul)
                    mps = ps_xt.tile([128, 512], F32, tag="sh")
                    nc.tensor.matmul(
                        out=mps[:, :],
                        lhsT=r32(wdiag_sb[ch][:, c, :]),
                        rhs=r32(xpad[:, 1 : N + 1]),
                        start=True,
                        stop=False,
                    )
                    nc.tensor.matmul(
                        out=mps[:, :],
                        lhsT=r32(brow_sb[ch][:, c * 128 : (c + 1) * 128]),
                        rhs=r32(ones512[:, :]),
                        start=False,
                        stop=True,
                    )
                    # the two shifted taps (DVE), first reads PSUM mid
                    mid = mid_pool.tile([128, N], F32)
                    nc.vector.scalar_tensor_tensor(
                        out=mid[:, :], in0=xpad[:, 0:N],
                        scalar=w_sb[:, c, 0:1], in1=mps[:, :],
                        op0=MUL, op1=ADD,
                    )
                    nc.vector.scalar_tensor_tensor(
                        out=r32(out_t[:, :]), in0=xpad[:, 2 : N + 2],
                        scalar=w_sb[:, c, 2:3], in1=mid[:, :],
                        op0=MUL, op1=ADD,
                    )
                qT.append(qt)
                kT.append(kt)
                vT.append(vt)

            # ---- v to token-major augmented tiles (ones col 0 -> r row 0) ----
            v_aug = []
            for jb in range(NJB):
                va = va_pool.tile([128, HEAD, D + 1], F32)
                for g in range(2):
                    vps = ps_xt.tile([128, 512], F32, tag="sh")
                    for kk in range(3):
                        cc = 3 * g + kk
                        nc.tensor.transpose(
                            out=r32(vps[:, kk * 128 : (kk + 1) * 128]),
                            in_=r32(vT[cc][:, jb * 128 : (jb + 1) * 128]),
                            identity=r32(ident_r[:, :]),
                        )
                    nc.scalar.copy(
                        out=r32(va[:, 6 * g : 6 * g + 6, 1 : D + 1]),
                        in_=vps[:, 0:384].rearrange("p (h d) -> p h d", h=6),
                    )
                nc.scalar.copy(
                    out=r32(va[:, :, 0:1]),
                    in_=ones_plain[:, 0:HEAD].rearrange("p (h o) -> p h o", o=1),
                )
                v_aug.append(va)

            # ---- attention per head pair ----
            attnT = []
            for pair in range(NCH):
                at = at_pool.tile([128, N], F32)
                for half in (0, 1):
                    h = 2 * pair + half
                    hp = slice(64 * half, 64 * half + 64)
                    exs = []
                    for sg in range(2):
                        sc = ps_sc.tile([128, 1024], F32, tag="sc")
                        for jj in range(2):
                            jb = 2 * sg + jj
                            nc.tensor.matmul(
                                out=sc[:, jj * 512 : (jj + 1) * 512],
                                lhsT=r32(kT[pair][hp, jb * 128 : (jb + 1) * 128]),
                                rhs=r32(qT[pair][hp, :]),
                                start=True,
                                stop=True,
                            )
                        ex = exp_pool.tile([128, 2, N], F32)
                        nc.scalar.activation(
                            out=r32(ex[:, :, :]),
                            in_=sc[:, :].rearrange("p (a b) -> p a b", a=2),
                            func=mybir.ActivationFunctionType.Exp,
                        )
                        exs.append(ex)
                    # attn^T rows 1..64; ones column makes row 0 = r
                    aps = ps_at.tile([D + 1, 512], F32)
                    for jc in range(NJB):
                        nc.tensor.matmul(
                            out=aps[:, :],
                            lhsT=r32(v_aug[jc][:, h, :]),
                            rhs=r32(exs[jc // 2][:, jc % 2, :]),
                            start=(jc == 0),
                            stop=(jc == NJB - 1),
                        )
                    # 1/r on partition 0, gpsimd broadcast, normalize, restack
                    rr = rr_pool.tile([1, 512], F32)
                    with nc.allow_low_precision(reason="softmax denominator"):
                        nc.vector.reciprocal(out=rr[:, :], in_=aps[0:1, :])
                    brc = brc_pool.tile([D + 1, 512], F32)
                    nc.gpsimd.partition_broadcast(brc[:, :], rr[:, :])
                    stk = st_pool.tile([D + 1, N], F32)
                    nc.vector.tensor_tensor(
                        out=r32(stk[:, :]), in0=aps[:, :],
                        in1=brc[:, :], op=MUL,
                    )
                    nc.sync.dma_start(
                        out=r32(at[64 * half : 64 * half + 64, :]),
                        in_=r32(stk[1 : D + 1, :]),
                    )
                attnT.append(at)

            # ---- output projection, token-major + bias, natural store ----
            for tb in range(NJB):
                ot = ot_pool.tile([128, FEAT], F32)
                for gh in range(2):
                    pj = ps_xt.tile([128, 512], F32, tag="sh")
                    gs = slice(gh * 384, gh * 384 + 384)
                    for fc in range(NCH):
                        nc.tensor.matmul(
                            out=pj[:, 0:384],
                            lhsT=r32(attnT[fc][:, tb * 128 : (tb + 1) * 128]),
                            rhs=r32(woT_sb[fc][:, gs]),
                            start=(fc == 0),
                            stop=False,
                        )
                    # += broadcast bias row (K=1 ones-row matmul)
                    nc.tensor.matmul(
                        out=pj[:, 0:384],
                        lhsT=r32(ones_row[:, :]),
                        rhs=r32(bo_row[:, gs]),
                        start=False,
                        stop=True,
                    )
                    nc.scalar.copy(out=ot[:, gs], in_=pj[:, 0:384])
                nc.sync.dma_start(
                    out=out_ap[b, tb * 128 : (tb + 1) * 128, :], in_=ot[:, :]
                )

    nc.compile()
    _PROG_CACHE["nc"] = nc
    return nc


def host_inputs(x, wq, bq, wk, bk, wv, bv, wo, bo):
    """Per-core input maps. Weight layout transforms + 1/sqrt(F) fold into q."""
    s = 1.0 / np.sqrt(np.float32(FEAT))

    def taps(w):  # (F,1,K) -> (128, NCH, K)
        return np.ascontiguousarray(
            w[:, 0, :].reshape(NCH, 128, KS).transpose(1, 0, 2)
        ).astype(np.float32)

    def cols(v):  # (F,) -> (128, NCH)
        return np.ascontiguousarray(v.reshape(NCH, 128).T).astype(np.float32)

    brow = np.stack([
        (bq * s).astype(np.float32), bk.astype(np.float32), bv.astype(np.float32)
    ]).reshape(3, FEAT)
    shared = {
        "wq": taps(wq) * s, "bq": cols(bq) * s,
        "wk": taps(wk), "bk": cols(bk),
        "wv": taps(wv), "bv": cols(bv),
        "brow": np.ascontiguousarray(brow).astype(np.float32),
        "woT": np.ascontiguousarray(wo.T).astype(np.float32),
        "bo": np.ascontiguousarray(bo.reshape(1, FEAT)).astype(np.float32),
    }
    return [
        {"x": np.ascontiguousarray(x[c * B_LOC : (c + 1) * B_LOC]).astype(np.float32),
         **shared}
        for c in range(NCORES)
    ]


def kernel(x, wq, bq, wk, bk, wv, bv, wo, bo):
    from concourse.bass_utils import run_bass_kernel_spmd

    nc = build_program()
    x = np.asarray(x)
    in_maps = host_inputs(
        x, np.asarray(wq), np.asarray(bq), np.asarray(wk), np.asarray(bk),
        np.asarray(wv), np.asarray(bv), np.asarray(wo), np.asarray(bo),
    )
    res = run_bass_kernel_spmd(nc, in_maps, list(range(NCORES)))
    out = np.concatenate([res.results[c]["out"] for c in range(NCORES)], axis=0)
    return out.astype(np.float32)
